# revision 19
# baseline (speedup 1.0000x reference)
"""AttentionHead kernel for Trainium2, 8 NeuronCores, data-parallel over batch.

Problem (fixed shapes):
    input_tensor [8, 2048, 1024] f32, attention_mask [8, 2048] int64 (0/1),
    Wq/Wk/Wv [1024, 128] f32, bq/bk/bv [128] f32.
    out = softmax(mask(Q @ K^T / sqrt(2048))) @ V    -> [8, 2048, 128] f32

Sharding: one batch element per core (B == n_cores == 8). No collectives.

Per-core device kernel (bf16 inputs, f32 accumulation). v2 design notes:
  - DMA order: wq, xt0, wk, xt1, bcol, mcol, xt2..xt7, wv so the PE can start
    the projection pipeline as soon as wq+xt0 land (~4.5us) instead of waiting
    for the whole 4MB X transfer.
  - Pass1 computes QT and KT chunk-by-chunk (DMA paced), pass2 computes VT
    from SBUF-resident X. PSUM is managed as 4 tags x 2 banks (q01/q23/
    k01/k23); each [128,1024] f32 slot holds two accumulation groups.
  - Mask handling is OFF the exp critical path entirely:
      * numerator: V rows are zeroed for masked keys during the V transpose
        copies (tensor_scalar_mul by the 0/1 mask column) -- free.
      * denominator: 16 matmuls per query block with lhsT = mask column
        ([128,1] 0/1 bf16), accumulating sum_j m_j^T E_j into PSUM row.
    So exp output feeds the PE directly; no DVE mask muls, no DVE tree.
  - Numerator [128,512] and denominator [1,512] share one 2-bank PSUM tile.
  - Normalize: reciprocal_approx_fast (5x faster than DVE reciprocal),
    gpsimd partition_broadcast, one DVE multiply, per-block out DMA.
  - Scores tiles ([128, 2*512] = one exp group) double-buffer through tags
    k01/k23; PE program order interleaves next-block scores between
    numerator/denominator matmul pairs so ScalarE (exp) stays saturated.
"""

import sys

for _p in ("/opt/trn_rl_repo", "/root/.axon_site/_ro/trn_rl_repo"):
    if _p not in sys.path:
        sys.path.append(_p)

import numpy as np
import ml_dtypes

B, S, DIN, DOUT = 8, 2048, 1024, 128
NCHUNK = DIN // 128          # 8 contraction chunks
NKEY = S // 128              # 16 key chunks
QBLK = 512                   # query block (free dim of S^T / OT matmuls)
NQB = S // QBLK              # 4 query blocks
STG = 2                      # key chunks per exp group ([128, STG*512] psum)
NGRP = NKEY // STG           # 8 exp groups per query block

BF16 = ml_dtypes.bfloat16


def _build():
    import concourse.bass as bass
    import concourse.tile as tile
    from concourse import bacc, mybir
    from concourse.masks import make_identity

    Alu = mybir.AluOpType

    f32 = mybir.dt.float32
    bf16 = mybir.dt.bfloat16
    Exp = mybir.ActivationFunctionType.Exp

    nc = bacc.Bacc("TRN2", target_bir_lowering=False, debug=False, num_devices=B)

    xt_d = nc.dram_tensor("xt", [DIN, S], bf16, kind="ExternalInput")
    wq_d = nc.dram_tensor("wq", [DIN, DOUT], bf16, kind="ExternalInput")
    wk_d = nc.dram_tensor("wk", [DIN, DOUT], bf16, kind="ExternalInput")
    wv_d = nc.dram_tensor("wv", [DIN, DOUT], bf16, kind="ExternalInput")
    bcol_d = nc.dram_tensor("bcol", [128, 3], f32, kind="ExternalInput")
    mcf_d = nc.dram_tensor("mcf", [128, NKEY], f32, kind="ExternalInput")
    out_d = nc.dram_tensor("out", [DOUT, S], f32, kind="ExternalOutput")

    with tile.TileContext(nc) as tc:
        with (
            tc.tile_pool(name="persist", bufs=1) as pp,
            tc.tile_pool(name="epool", bufs=4) as ep,
            tc.tile_pool(name="normp", bufs=2) as rp,
            tc.tile_pool(name="outp", bufs=2) as op,
            tc.tile_pool(name="psum", bufs=1, space="PSUM") as ps,
        ):
            xts = [pp.tile([128, S], bf16, tag=f"xt{c}", name=f"xt{c}")
                   for c in range(NCHUNK)]
            wq = pp.tile([128, NCHUNK * DOUT], bf16, tag="wq")
            wk = pp.tile([128, NCHUNK * DOUT], bf16, tag="wk")
            wv = pp.tile([128, NCHUNK * DOUT], bf16, tag="wv")
            bcol = pp.tile([128, 3], f32, tag="bcol")
            mcf = pp.tile([128, NKEY], f32, tag="mcf")
            ident = pp.tile([128, 128], bf16, tag="ident")
            ocol = pp.tile([128, 1], bf16, tag="ocol")
            qt = pp.tile([128, S], bf16, tag="qt")
            kt = pp.tile([128, S], bf16, tag="kt")
            vt = pp.tile([128, S], bf16, tag="vt")
            vn = pp.tile([128, NKEY * 128], bf16, tag="vn")

            # ---- DMA issue order: wq, xt0 (split), wk, xt1, bcol, mcol, ...
            xt3 = xt_d.ap().rearrange("(c p) m -> p c m", p=128)
            wq4 = wq_d.ap().rearrange("(c p) e -> p c e", p=128)
            wk4 = wk_d.ap().rearrange("(c p) e -> p c e", p=128)
            wqs = wq[:].rearrange("p (c e) -> p c e", c=NCHUNK)
            wks = wk[:].rearrange("p (c e) -> p c e", c=NCHUNK)
            nc.sync.dma_start(wqs[:, 0:1], wq4[:, 0:1])
            nc.sync.dma_start(wks[:, 0:1], wk4[:, 0:1])
            nc.sync.dma_start(xts[0][:], xt3[:, 0, :])
            nc.sync.dma_start(wqs[:, 1:], wq4[:, 1:])
            nc.sync.dma_start(wks[:, 1:], wk4[:, 1:])
            nc.sync.dma_start(xts[1][:], xt3[:, 1, :])
            nc.sync.dma_start(bcol[:], bcol_d.ap())
            nc.sync.dma_start(mcf[:], mcf_d.ap())
            for c in range(2, NCHUNK):
                nc.sync.dma_start(xts[c][:], xt3[:, c, :])
            nc.sync.dma_start(wv[:].rearrange("p (c e) -> p c e", c=NCHUNK),
                              wv_d.ap().rearrange("(c p) e -> p c e", p=128))
            mcolb = pp.tile([128, NKEY], bf16, tag="mcolb")
            make_identity(nc, ident[:])
            nc.vector.memset(ocol[:], 1.0)
            nc.vector.tensor_copy(mcolb[:], mcf[:])

            # PSUM slots: 4 tags x [128,1024] f32 (2 banks each).
            def pslot(tag, cyc, shape=None, dtype=f32):
                return ps.tile(shape or [128, 2 * QBLK], dtype, tag=tag,
                               name=f"{tag}_c{cyc}")

            # ---- Phase A pass1: QT, KT accumulation (DMA paced) ----
            pq = [pslot("q01", 0), pslot("q23", 0)]   # q t0/t1, t2/t3
            pk = [pslot("k01", 0), pslot("k23", 0)]
            for c in range(NCHUNK):
                st, sp = (c == 0), (c == NCHUNK - 1)
                for t in range(NQB):
                    nc.tensor.matmul(
                        pq[t // 2][:, (t % 2) * QBLK:(t % 2 + 1) * QBLK],
                        wq[:, c * DOUT:(c + 1) * DOUT],
                        xts[c][:, t * QBLK:(t + 1) * QBLK],
                        start=st, stop=sp,
                    )
                for t in range(NQB):
                    nc.tensor.matmul(
                        pk[t // 2][:, (t % 2) * QBLK:(t % 2 + 1) * QBLK],
                        wk[:, c * DOUT:(c + 1) * DOUT],
                        xts[c][:, t * QBLK:(t + 1) * QBLK],
                        start=st, stop=sp,
                    )

            # PSUM -> SBUF copies with bias add on ScalarE (idle until the
            # first exp); emission is interleaved with scores() calls below so
            # each drain lands just before the exp that needs it.
            def drain(dst, src, bc, t, eng="scalar"):
                d_ = dst[:, t * QBLK:(t + 1) * QBLK]
                s_ = src[t // 2][:, (t % 2) * QBLK:(t % 2 + 1) * QBLK]
                if eng == "scalar":
                    nc.scalar.add(d_, s_, bcol[:, bc:bc + 1])
                else:
                    nc.vector.tensor_scalar_add(d_, s_, bcol[:, bc:bc + 1])

            drain(kt, pk, 1, 0, "vec")
            drain(qt, pq, 0, 0, "vec")
            drain(kt, pk, 1, 1, "vec")

            # ---- Phase C state ----
            egs = {}           # (t, g) -> exp tile
            ktag = ["k01", "k23"]
            kcyc = [1, 1]
            sgi = [0]          # global scores-group index for tag alternation

            def scores(t, g):
                i = sgi[0] % 2
                pst = pslot(ktag[i], kcyc[i])
                kcyc[i] += 1
                sgi[0] += 1
                for jj in range(STG):
                    j = g * STG + jj
                    nc.tensor.matmul(
                        pst[:, jj * QBLK:(jj + 1) * QBLK],
                        kt[:, j * 128:(j + 1) * 128],
                        qt[:, t * QBLK:(t + 1) * QBLK],
                        start=True, stop=True,
                    )
                eg = ep.tile([128, STG * QBLK], bf16, tag="e",
                             name=f"eg{t}_{g}", bufs=10)
                nc.scalar.activation(eg[:], pst[:], Exp)
                egs[(t, g)] = eg

            # ---- Phase A pass2: VT (SBUF resident, t-outer so each t-block
            # drains early), interleaved with the first query block's scores
            # so ScalarE starts early and the PE never idles into phase B.
            pv = [pslot("q01", 1), pslot("q23", 1)]
            scores(0, 0)
            drain(kt, pk, 1, 2, "vec")
            drain(kt, pk, 1, 3, "vec")
            scores(0, 1)

            def vproj(t):
                for c in range(NCHUNK):
                    nc.tensor.matmul(
                        pv[t // 2][:, (t % 2) * QBLK:(t % 2 + 1) * QBLK],
                        wv[:, c * DOUT:(c + 1) * DOUT],
                        xts[c][:, t * QBLK:(t + 1) * QBLK],
                        start=(c == 0), stop=(c == NCHUNK - 1),
                    )

            vproj(0)
            drain(qt, pq, 0, 1, "vec")
            scores(0, 2)
            vproj(1)
            drain(vt, pv, 2, 0, "vec")
            drain(vt, pv, 2, 1, "vec")
            drain(qt, pq, 0, 2, "vec")
            drain(qt, pq, 0, 3, "vec")
            scores(0, 3)
            vproj(2)
            scores(0, 4)
            vproj(3)
            drain(vt, pv, 2, 2, "vec")
            drain(vt, pv, 2, 3, "vec")
            scores(0, 5)
            # ---- Phase B: V natural layout via PE transpose, mask folded
            # into the PSUM->SBUF copy (zero masked key rows of V).
            qtag = ["q01", "q23"]
            qcyc = [2, 2]

            def vtrans(kb):
                i = kb // 2
                ptr = pslot(qtag[i], qcyc[i], shape=[128, 4 * 128], dtype=bf16)
                qcyc[i] += 1
                for jj in range(4):
                    j = kb * 4 + jj
                    nc.tensor.transpose(ptr[:, jj * 128:(jj + 1) * 128],
                                        vt[:, j * 128:(j + 1) * 128],
                                        ident[:])
                for jj in range(4):
                    j = kb * 4 + jj
                    nc.vector.tensor_scalar_mul(
                        vn[:, j * 128:(j + 1) * 128],
                        ptr[:, jj * 128:(jj + 1) * 128],
                        mcf[:, j:j + 1])

            scores(0, 6)
            vtrans(0)
            scores(0, 7)
            vtrans(1)
            vtrans(2)
            vtrans(3)

            # ---- Phase C main loop ----
            ods = {}
            dss = {}

            def numer(t, j):
                od = ods[t]
                g, jj = j // STG, j % STG
                eg = egs[(t, g)]
                nc.tensor.matmul(
                    od[:, 0:QBLK],
                    vn[:, j * 128:(j + 1) * 128],
                    eg[:, jj * QBLK:(jj + 1) * QBLK],
                    start=(j == 0), stop=(j == NKEY - 1),
                )

            def dchain(t, j):
                # masked running sum of exp: low chain on DVE, high on gpsimd
                g, jj = j // STG, j % STG
                sl = egs[(t, g)][:, jj * QBLK:(jj + 1) * QBLK]
                h, hj = j // 8, j % 8
                eng = nc.vector
                s = dss[(t, h)]
                if hj == 0:
                    eng.tensor_scalar_mul(s[:], sl, mcf[:, j:j + 1])
                else:
                    eng.scalar_tensor_tensor(
                        s[:], sl, mcf[:, j:j + 1], s[:],
                        op0=Alu.mult, op1=Alu.add)
                if j == NKEY - 1:
                    nc.vector.tensor_add(s[:], s[:], dss[(t, 0)][:])

            rdbs = {}

            def dmm(t, j):
                # denominator chunk on the PE (mask column as weights)
                g, jj = j // STG, j % STG
                nc.tensor.matmul(
                    ods[t][0:1, QBLK:2 * QBLK],
                    mcolb[:, j:j + 1],
                    egs[(t, g)][:, jj * QBLK:(jj + 1) * QBLK],
                    start=(j == 0), stop=(j == NKEY - 1),
                )

            def finish_a(t):
                # denominator matmul (PE) then reciprocal + broadcast
                od = ods[t]
                if t < NQB - 1:
                    nc.tensor.matmul(od[0:1, QBLK:2 * QBLK], ocol[:],
                                     dss[(t, 1)][:], start=True, stop=True)
                rd = rp.tile([1, QBLK], f32, tag="rd", name=f"rd{t}")
                nc.vector.reciprocal_approx_fast(rd[:], od[0:1, QBLK:2 * QBLK])
                rdb = rp.tile([128, QBLK], f32, tag="rdb", name=f"rdb{t}")
                nc.gpsimd.partition_broadcast(rdb[:], rd[:])
                rdbs[t] = rdb

            def finish_b(t):
                osb = op.tile([128, QBLK], f32, tag="osb", name=f"osb{t}")
                nc.vector.tensor_mul(osb[:], ods[t][:, 0:QBLK], rdbs[t][:])
                nc.sync.dma_start(out_d.ap()[:, t * QBLK:(t + 1) * QBLK],
                                  osb[:])

            for t in range(NQB):
                i = t % 2
                ods[t] = pslot(qtag[i], qcyc[i])
                qcyc[i] += 1
                if t < NQB - 1:
                    dss[(t, 0)] = rp.tile([128, QBLK], bf16, tag=f"ds0_{i}",
                                          name=f"ds0_{t}")
                    dss[(t, 1)] = rp.tile([128, QBLK], bf16, tag=f"ds1_{i}",
                                          name=f"ds1_{t}")
                last = (t == NQB - 1)
                for g in range(NGRP):
                    if not last:
                        scores(t + 1, g)
                        numer(t, STG * g)
                        numer(t, STG * g + 1)
                        dchain(t, STG * g)
                        dchain(t, STG * g + 1)
                    else:
                        dmm(t, STG * g)
                        dmm(t, STG * g + 1)
                        numer(t, STG * g)
                        numer(t, STG * g + 1)
                    if g == 0 and t > 0:
                        finish_b(t - 1)
                finish_a(t)
            finish_b(NQB - 1)

    nc.compile()
    return nc


_NC = None


def _get_nc():
    global _NC
    if _NC is None:
        _NC = _build()
    return _NC


def _prep_in_maps(input_tensor, attention_mask, Wq, bq, Wk, bk, Wv, bv):
    scale = np.float32(1.0 / np.sqrt(np.float32(S)))
    wq_h = (np.asarray(Wq, np.float32) * scale).astype(BF16)
    wk_h = np.asarray(Wk, np.float32).astype(BF16)
    wv_h = np.asarray(Wv, np.float32).astype(BF16)
    bcol_h = np.stack(
        [np.asarray(bq, np.float32) * scale,
         np.asarray(bk, np.float32),
         np.asarray(bv, np.float32)], axis=1).astype(np.float32)  # [128,3]

    x = np.asarray(input_tensor, np.float32)
    m = np.asarray(attention_mask)
    in_maps = []
    for b in range(B):
        xt_h = np.ascontiguousarray(x[b].T).astype(BF16)            # [DIN, S]
        mcf_h = np.ascontiguousarray(
            m[b].astype(np.float32).reshape(NKEY, 128).T)
        in_maps.append({
            "xt": xt_h, "wq": wq_h, "wk": wk_h, "wv": wv_h,
            "bcol": bcol_h, "mcf": mcf_h,
        })
    return in_maps


def run(in_maps, trace=False, **kwargs):
    from concourse.bass_utils import run_bass_kernel_spmd

    nc = _get_nc()
    return run_bass_kernel_spmd(
        nc, in_maps, core_ids=list(range(B)), trace=trace, **kwargs
    )


def kernel(input_tensor, attention_mask, Wq, bq, Wk, bk, Wv, bv):
    in_maps = _prep_in_maps(
        input_tensor, attention_mask, Wq, bq, Wk, bk, Wv, bv)
    res = run(in_maps, trace=False)
    out = np.stack([res.results[b]["out"].T for b in range(B)])
    return np.ascontiguousarray(out.astype(np.float32))


# revision 20
# speedup vs baseline: 1.0093x; 1.0093x over previous
"""AttentionHead kernel for Trainium2, 8 NeuronCores, data-parallel over batch.

Problem (fixed shapes):
    input_tensor [8, 2048, 1024] f32, attention_mask [8, 2048] int64 (0/1),
    Wq/Wk/Wv [1024, 128] f32, bq/bk/bv [128] f32.
    out = softmax(mask(Q @ K^T / sqrt(2048))) @ V    -> [8, 2048, 128] f32

Sharding: one batch element per core (B == n_cores == 8). No collectives.

Per-core device kernel (bf16 inputs, f32 accumulation). v2 design notes:
  - DMA order: wq, xt0, wk, xt1, bcol, mcol, xt2..xt7, wv so the PE can start
    the projection pipeline as soon as wq+xt0 land (~4.5us) instead of waiting
    for the whole 4MB X transfer.
  - Pass1 computes QT and KT chunk-by-chunk (DMA paced), pass2 computes VT
    from SBUF-resident X. PSUM is managed as 4 tags x 2 banks (q01/q23/
    k01/k23); each [128,1024] f32 slot holds two accumulation groups.
  - Mask handling is OFF the exp critical path entirely:
      * numerator: V rows are zeroed for masked keys during the V transpose
        copies (tensor_scalar_mul by the 0/1 mask column) -- free.
      * denominator: 16 matmuls per query block with lhsT = mask column
        ([128,1] 0/1 bf16), accumulating sum_j m_j^T E_j into PSUM row.
    So exp output feeds the PE directly; no DVE mask muls, no DVE tree.
  - Numerator [128,512] and denominator [1,512] share one 2-bank PSUM tile.
  - Normalize: reciprocal_approx_fast (5x faster than DVE reciprocal),
    gpsimd partition_broadcast, one DVE multiply, per-block out DMA.
  - Scores tiles ([128, 2*512] = one exp group) double-buffer through tags
    k01/k23; PE program order interleaves next-block scores between
    numerator/denominator matmul pairs so ScalarE (exp) stays saturated.
"""

import sys

for _p in ("/opt/trn_rl_repo", "/root/.axon_site/_ro/trn_rl_repo"):
    if _p not in sys.path:
        sys.path.append(_p)

import numpy as np
import ml_dtypes

B, S, DIN, DOUT = 8, 2048, 1024, 128
NCHUNK = DIN // 128          # 8 contraction chunks
NKEY = S // 128              # 16 key chunks
QBLK = 512                   # query block (free dim of S^T / OT matmuls)
NQB = S // QBLK              # 4 query blocks
STG = 2                      # key chunks per exp group ([128, STG*512] psum)
NGRP = NKEY // STG           # 8 exp groups per query block

BF16 = ml_dtypes.bfloat16


def _build():
    import concourse.bass as bass
    import concourse.tile as tile
    from concourse import bacc, mybir
    from concourse.masks import make_identity

    Alu = mybir.AluOpType

    f32 = mybir.dt.float32
    bf16 = mybir.dt.bfloat16
    Exp = mybir.ActivationFunctionType.Exp

    nc = bacc.Bacc("TRN2", target_bir_lowering=False, debug=False, num_devices=B)

    xt_d = nc.dram_tensor("xt", [DIN, S], bf16, kind="ExternalInput")
    wq_d = nc.dram_tensor("wq", [DIN, DOUT], bf16, kind="ExternalInput")
    wk_d = nc.dram_tensor("wk", [DIN, DOUT], bf16, kind="ExternalInput")
    wv_d = nc.dram_tensor("wv", [DIN, DOUT], bf16, kind="ExternalInput")
    bcol_d = nc.dram_tensor("bcol", [128, 3], f32, kind="ExternalInput")
    mcf_d = nc.dram_tensor("mcf", [128, NKEY], f32, kind="ExternalInput")
    out_d = nc.dram_tensor("out", [DOUT, S], f32, kind="ExternalOutput")

    with tile.TileContext(nc) as tc:
        with (
            tc.tile_pool(name="persist", bufs=1) as pp,
            tc.tile_pool(name="epool", bufs=4) as ep,
            tc.tile_pool(name="normp", bufs=2) as rp,
            tc.tile_pool(name="outp", bufs=2) as op,
            tc.tile_pool(name="psum", bufs=1, space="PSUM") as ps,
        ):
            xts = [pp.tile([128, S], bf16, tag=f"xt{c}", name=f"xt{c}")
                   for c in range(NCHUNK)]
            wq = pp.tile([128, NCHUNK * DOUT], bf16, tag="wq")
            wk = pp.tile([128, NCHUNK * DOUT], bf16, tag="wk")
            wv = pp.tile([128, NCHUNK * DOUT], bf16, tag="wv")
            bcol = pp.tile([128, 3], f32, tag="bcol")
            mcf = pp.tile([128, NKEY], f32, tag="mcf")
            ident = pp.tile([128, 128], bf16, tag="ident")
            ocol = pp.tile([128, 1], bf16, tag="ocol")
            qt = pp.tile([128, S], bf16, tag="qt")
            kt = pp.tile([128, S], bf16, tag="kt")
            vt = pp.tile([128, S], bf16, tag="vt")
            vn = pp.tile([128, NKEY * 128], bf16, tag="vn")

            # ---- DMA issue order: wq, xt0 (split), wk, xt1, bcol, mcol, ...
            xt3 = xt_d.ap().rearrange("(c p) m -> p c m", p=128)
            wq4 = wq_d.ap().rearrange("(c p) e -> p c e", p=128)
            wk4 = wk_d.ap().rearrange("(c p) e -> p c e", p=128)
            wqs = wq[:].rearrange("p (c e) -> p c e", c=NCHUNK)
            wks = wk[:].rearrange("p (c e) -> p c e", c=NCHUNK)
            nc.sync.dma_start(wqs[:, 0:1], wq4[:, 0:1])
            nc.sync.dma_start(wks[:, 0:1], wk4[:, 0:1])
            nc.sync.dma_start(xts[0][:], xt3[:, 0, :])
            nc.sync.dma_start(wqs[:, 1:], wq4[:, 1:])
            nc.sync.dma_start(wks[:, 1:], wk4[:, 1:])
            nc.sync.dma_start(xts[1][:], xt3[:, 1, :])
            nc.sync.dma_start(bcol[:], bcol_d.ap())
            nc.sync.dma_start(mcf[:], mcf_d.ap())
            for c in range(2, NCHUNK):
                nc.sync.dma_start(xts[c][:], xt3[:, c, :])
            nc.sync.dma_start(wv[:].rearrange("p (c e) -> p c e", c=NCHUNK),
                              wv_d.ap().rearrange("(c p) e -> p c e", p=128))
            mcolb = pp.tile([128, NKEY], bf16, tag="mcolb")
            make_identity(nc, ident[:])
            nc.vector.memset(ocol[:], 1.0)
            nc.vector.tensor_copy(mcolb[:], mcf[:])

            # PSUM slots: 4 tags x [128,1024] f32 (2 banks each).
            def pslot(tag, cyc, shape=None, dtype=f32):
                return ps.tile(shape or [128, 2 * QBLK], dtype, tag=tag,
                               name=f"{tag}_c{cyc}")

            # ---- Phase A pass1: QT, KT accumulation (DMA paced) ----
            pq = [pslot("q01", 0), pslot("q23", 0)]   # q t0/t1, t2/t3
            pk = [pslot("k01", 0), pslot("k23", 0)]
            for c in range(NCHUNK):
                st, sp = (c == 0), (c == NCHUNK - 1)
                for t in range(NQB):
                    nc.tensor.matmul(
                        pq[t // 2][:, (t % 2) * QBLK:(t % 2 + 1) * QBLK],
                        wq[:, c * DOUT:(c + 1) * DOUT],
                        xts[c][:, t * QBLK:(t + 1) * QBLK],
                        start=st, stop=sp,
                    )
                for t in range(NQB):
                    nc.tensor.matmul(
                        pk[t // 2][:, (t % 2) * QBLK:(t % 2 + 1) * QBLK],
                        wk[:, c * DOUT:(c + 1) * DOUT],
                        xts[c][:, t * QBLK:(t + 1) * QBLK],
                        start=st, stop=sp,
                    )

            # PSUM -> SBUF copies with bias add on ScalarE (idle until the
            # first exp); emission is interleaved with scores() calls below so
            # each drain lands just before the exp that needs it.
            def drain(dst, src, bc, t, eng="scalar"):
                d_ = dst[:, t * QBLK:(t + 1) * QBLK]
                s_ = src[t // 2][:, (t % 2) * QBLK:(t % 2 + 1) * QBLK]
                if eng == "scalar":
                    nc.scalar.add(d_, s_, bcol[:, bc:bc + 1])
                else:
                    nc.vector.tensor_scalar_add(d_, s_, bcol[:, bc:bc + 1])

            drain(kt, pk, 1, 0, "vec")
            drain(qt, pq, 0, 0, "vec")
            drain(kt, pk, 1, 1, "vec")

            # ---- Phase C state ----
            egs = {}           # (t, g) -> exp tile
            ktag = ["k01", "k23"]
            kcyc = [1, 1]
            sgi = [0]          # global scores-group index for tag alternation

            def scores(t, g):
                i = sgi[0] % 2
                pst = pslot(ktag[i], kcyc[i])
                kcyc[i] += 1
                sgi[0] += 1
                for jj in range(STG):
                    j = g * STG + jj
                    nc.tensor.matmul(
                        pst[:, jj * QBLK:(jj + 1) * QBLK],
                        kt[:, j * 128:(j + 1) * 128],
                        qt[:, t * QBLK:(t + 1) * QBLK],
                        start=True, stop=True,
                    )
                eg = ep.tile([128, STG * QBLK], bf16, tag="e",
                             name=f"eg{t}_{g}", bufs=10)
                nc.scalar.activation(eg[:], pst[:], Exp)
                egs[(t, g)] = eg

            # ---- Phase A pass2: VT (SBUF resident, t-outer so each t-block
            # drains early), interleaved with the first query block's scores
            # so ScalarE starts early and the PE never idles into phase B.
            pv = [pslot("q01", 1), pslot("q23", 1)]
            scores(0, 0)
            drain(kt, pk, 1, 2, "vec")
            drain(kt, pk, 1, 3, "vec")
            scores(0, 1)

            def vproj(t):
                for c in range(NCHUNK):
                    nc.tensor.matmul(
                        pv[t // 2][:, (t % 2) * QBLK:(t % 2 + 1) * QBLK],
                        wv[:, c * DOUT:(c + 1) * DOUT],
                        xts[c][:, t * QBLK:(t + 1) * QBLK],
                        start=(c == 0), stop=(c == NCHUNK - 1),
                    )

            vproj(0)
            drain(qt, pq, 0, 1, "vec")
            scores(0, 2)
            vproj(1)
            drain(vt, pv, 2, 0, "vec")
            drain(vt, pv, 2, 1, "vec")
            drain(qt, pq, 0, 2, "vec")
            drain(qt, pq, 0, 3, "vec")
            scores(0, 3)
            vproj(2)
            scores(0, 4)
            vproj(3)
            drain(vt, pv, 2, 2, "vec")
            drain(vt, pv, 2, 3, "vec")
            scores(0, 5)
            # ---- Phase B: V natural layout via PE transpose, mask folded
            # into the PSUM->SBUF copy (zero masked key rows of V).
            qtag = ["q01", "q23"]
            qcyc = [2, 2]

            def vtrans(kb):
                i = kb // 2
                ptr = pslot(qtag[i], qcyc[i], shape=[128, 4 * 128], dtype=bf16)
                qcyc[i] += 1
                for jj in range(4):
                    j = kb * 4 + jj
                    nc.tensor.transpose(ptr[:, jj * 128:(jj + 1) * 128],
                                        vt[:, j * 128:(j + 1) * 128],
                                        ident[:])
                for jj in range(4):
                    j = kb * 4 + jj
                    nc.vector.tensor_scalar_mul(
                        vn[:, j * 128:(j + 1) * 128],
                        ptr[:, jj * 128:(jj + 1) * 128],
                        mcf[:, j:j + 1])

            scores(0, 6)
            vtrans(0)
            scores(0, 7)
            vtrans(1)
            vtrans(2)
            vtrans(3)

            # ---- Phase C main loop ----
            ods = {}
            dss = {}

            def numer(t, j):
                od = ods[t]
                g, jj = j // STG, j % STG
                eg = egs[(t, g)]
                nc.tensor.matmul(
                    od[:, 0:QBLK],
                    vn[:, j * 128:(j + 1) * 128],
                    eg[:, jj * QBLK:(jj + 1) * QBLK],
                    start=(j == 0), stop=(j == NKEY - 1),
                )

            def dchain(t, j):
                # masked running sum of exp: low chain on DVE, high on gpsimd
                g, jj = j // STG, j % STG
                sl = egs[(t, g)][:, jj * QBLK:(jj + 1) * QBLK]
                if j >= NKEY - 2:
                    return          # last two chunks go to the PE (finish_a)
                h, hj = j // 8, j % 8
                s = dss[(t, h)]
                if hj == 0:
                    nc.vector.tensor_scalar_mul(s[:], sl, mcf[:, j:j + 1])
                else:
                    nc.vector.scalar_tensor_tensor(
                        s[:], sl, mcf[:, j:j + 1], s[:],
                        op0=Alu.mult, op1=Alu.add)
                if j == NKEY - 3:
                    nc.vector.tensor_add(s[:], s[:], dss[(t, 0)][:])

            rdbs = {}

            def dmm(t, j, start=None, stop=None):
                # denominator chunk on the PE (mask column as weights)
                g, jj = j // STG, j % STG
                nc.tensor.matmul(
                    ods[t][0:1, QBLK:2 * QBLK],
                    mcolb[:, j:j + 1],
                    egs[(t, g)][:, jj * QBLK:(jj + 1) * QBLK],
                    start=(j == 0) if start is None else start,
                    stop=(j == NKEY - 1) if stop is None else stop,
                )

            def finish_a(t):
                # denominator: last two chunks + DVE-chain total on the PE
                od = ods[t]
                if t < NQB - 1:
                    dmm(t, NKEY - 2, start=True, stop=False)
                    dmm(t, NKEY - 1, start=False, stop=False)
                    nc.tensor.matmul(od[0:1, QBLK:2 * QBLK], ocol[:],
                                     dss[(t, 1)][:], start=False, stop=True)
                rd = rp.tile([1, QBLK], f32, tag="rd", name=f"rd{t}")
                nc.vector.reciprocal_approx_fast(rd[:], od[0:1, QBLK:2 * QBLK])
                rdb = rp.tile([128, QBLK], f32, tag="rdb", name=f"rdb{t}")
                nc.gpsimd.partition_broadcast(rdb[:], rd[:])
                rdbs[t] = rdb

            def finish_b(t):
                osb = op.tile([128, QBLK], f32, tag="osb", name=f"osb{t}")
                nc.vector.tensor_mul(osb[:], ods[t][:, 0:QBLK], rdbs[t][:])
                nc.sync.dma_start(out_d.ap()[:, t * QBLK:(t + 1) * QBLK],
                                  osb[:])

            for t in range(NQB):
                i = t % 2
                ods[t] = pslot(qtag[i], qcyc[i])
                qcyc[i] += 1
                if t < NQB - 1:
                    dss[(t, 0)] = rp.tile([128, QBLK], bf16, tag=f"ds0_{i}",
                                          name=f"ds0_{t}")
                    dss[(t, 1)] = rp.tile([128, QBLK], bf16, tag=f"ds1_{i}",
                                          name=f"ds1_{t}")
                last = (t == NQB - 1)
                for g in range(NGRP):
                    if not last:
                        scores(t + 1, g)
                        numer(t, STG * g)
                        numer(t, STG * g + 1)
                        dchain(t, STG * g)
                        dchain(t, STG * g + 1)
                    else:
                        dmm(t, STG * g)
                        dmm(t, STG * g + 1)
                        numer(t, STG * g)
                        numer(t, STG * g + 1)
                    if g == 0 and t > 0:
                        finish_b(t - 1)
                finish_a(t)
            finish_b(NQB - 1)

    nc.compile()
    return nc


_NC = None


def _get_nc():
    global _NC
    if _NC is None:
        _NC = _build()
    return _NC


def _prep_in_maps(input_tensor, attention_mask, Wq, bq, Wk, bk, Wv, bv):
    scale = np.float32(1.0 / np.sqrt(np.float32(S)))
    wq_h = (np.asarray(Wq, np.float32) * scale).astype(BF16)
    wk_h = np.asarray(Wk, np.float32).astype(BF16)
    wv_h = np.asarray(Wv, np.float32).astype(BF16)
    bcol_h = np.stack(
        [np.asarray(bq, np.float32) * scale,
         np.asarray(bk, np.float32),
         np.asarray(bv, np.float32)], axis=1).astype(np.float32)  # [128,3]

    x = np.asarray(input_tensor, np.float32)
    m = np.asarray(attention_mask)
    in_maps = []
    for b in range(B):
        xt_h = np.ascontiguousarray(x[b].T).astype(BF16)            # [DIN, S]
        mcf_h = np.ascontiguousarray(
            m[b].astype(np.float32).reshape(NKEY, 128).T)
        in_maps.append({
            "xt": xt_h, "wq": wq_h, "wk": wk_h, "wv": wv_h,
            "bcol": bcol_h, "mcf": mcf_h,
        })
    return in_maps


def run(in_maps, trace=False, **kwargs):
    from concourse.bass_utils import run_bass_kernel_spmd

    nc = _get_nc()
    return run_bass_kernel_spmd(
        nc, in_maps, core_ids=list(range(B)), trace=trace, **kwargs
    )


def kernel(input_tensor, attention_mask, Wq, bq, Wk, bk, Wv, bv):
    in_maps = _prep_in_maps(
        input_tensor, attention_mask, Wq, bq, Wk, bk, Wv, bv)
    res = run(in_maps, trace=False)
    out = np.stack([res.results[b]["out"].T for b in range(B)])
    return np.ascontiguousarray(out.astype(np.float32))
